# revision 74
# baseline (speedup 1.0000x reference)
"""Trainium2 Bass kernel for one GPT-style transformer block.

Problem: x[8,1024,1024]; per-core = one batch element (data-parallel over 8
NeuronCores).  Per core:
    h1 = LN(x); qkv = h1@Wqkv+b; causal MHA (16 heads, d=64);
    r1 = x + attn@Wproj+b; h2 = LN(r1); out = r1 + relu(h2@W1+b1)@W2+b2

v2 (all-bf16): v1 measured 600 us/core with PE busy 517 us; 1800 of 2248
matmuls ran with fp32 weights (LDWEIGHTS 187 ns vs 97 bf16; issue spacing
234 vs 215 ns) and 196 fp32 PE transposes at ~270 ns.  v2 moves every
matmul operand to bf16 (numpy sim: rel err 3.5e-3 vs 2e-2 budget):
  - weights folded (LN gains into Wqkv/W1) then cast bf16 and repacked
    tile-major on the host so each weight tile is one contiguous DMA.
  - activations h1/y/h2/a1 and the residual xf all bf16; LN stats and
    PSUM accumulation stay f32.
  - all PE transposes bf16 (x is cast to bf16 on gpsimd first); each
    128x1024 tile transposes through one 2KB PSUM bank (8 mms + 1 evict).
  - x feature-major transposes are deferred into the attention loop as
    PE filler; LN1 applies split ACT/DVE; casts on gpsimd.
  - bproj/b2 residual pre-adds on gpsimd; LN2 apply split DVE (t=0) /
    gpsimd (t=1); Wproj/W1-qtr0 prefetched during attention.
  - FFN2 out-path: 8 transposes + 1 wide evict per chunk, out-DMA split
    per 512-token half.
"""

import math
import sys

import numpy as np

sys.path.insert(0, "/opt/trn_rl_repo")

from contextlib import ExitStack

import concourse.bass as bass
import concourse.mybir as mybir
import concourse.tile as tile
from concourse import bacc
from concourse.bass import ts
from concourse.masks import make_identity

F32 = mybir.dt.float32
F32R = mybir.dt.float32r
BF16 = mybir.dt.bfloat16
FP8 = mybir.dt.float8e4
DR = mybir.MatmulPerfMode.DoubleRow
SA = 16.0           # fixed activation quant scale for h1 (LN output, |h|<~7)
AF = mybir.ActivationFunctionType
ALU = mybir.AluOpType
AX = mybir.AxisListType

B, T, C, H = 8, 1024, 1024, 16
D = C // H
FF = 4 * C
P = 128
NCH = C // P          # 8 feature chunks
NT = T // P           # 8 token chunks of 128
NQ = T // 512         # 2 token chunks of 512
SCALE = 1.0 / math.sqrt(3 * C // H)
EPS = 1e-5


def _build():
    nc = bacc.Bacc("TRN2", target_bir_lowering=False, debug=False)

    x_d = nc.dram_tensor("x", [T, C], F32R, kind="ExternalInput").ap()
    # weights are host-repacked tile-major: tile ti = [128 kpart, kchunks, 128]
    # Wqkv is fp8e4 with DoubleRow k-pair layout [128, 4 pairs, 2, 128]
    Wqkv_d = nc.dram_tensor("Wqkv", [3 * NCH, P, NCH // 2, 2, P], FP8,
                            kind="ExternalInput").ap()
    bqkv_d = nc.dram_tensor("bqkv", [3 * C], F32, kind="ExternalInput").ap()
    qdq_d = nc.dram_tensor("qdq", [3 * C], F32, kind="ExternalInput").ap()
    Wproj_d = nc.dram_tensor("Wproj", [NCH, P, NCH, P], BF16,
                             kind="ExternalInput").ap()
    bproj_d = nc.dram_tensor("bproj", [C], F32, kind="ExternalInput").ap()
    W1_d = nc.dram_tensor("W1", [FF // P, P, NCH, P], BF16,
                          kind="ExternalInput").ap()
    b1_d = nc.dram_tensor("b1", [FF], F32, kind="ExternalInput").ap()
    W2_d = nc.dram_tensor("W2", [FF // P, P, 8, P], BF16,
                          kind="ExternalInput").ap()
    b2_d = nc.dram_tensor("b2", [C], F32, kind="ExternalInput").ap()
    out_d = nc.dram_tensor("out", [T, C], F32, kind="ExternalOutput").ap()

    with nc.allow_low_precision(reason="bf16 matmul inputs (fp32 accum)"), \
         tile.TileContext(nc) as tc, ExitStack() as ctx:
        const = ctx.enter_context(tc.tile_pool(name="const", bufs=1))
        xpool = ctx.enter_context(tc.tile_pool(name="xpool", bufs=1))
        hpool = ctx.enter_context(tc.tile_pool(name="hpool", bufs=1))
        spool = ctx.enter_context(tc.tile_pool(name="spool", bufs=2))
        wpool = ctx.enter_context(tc.tile_pool(name="wpool", bufs=2))
        ffnp = ctx.enter_context(tc.tile_pool(name="ffnp", bufs=1))
        blp = ctx.enter_context(tc.tile_pool(name="bload", bufs=1))
        qkv_ctx = ExitStack()   # closed just before the FFN block
        qkvp = qkv_ctx.enter_context(tc.tile_pool(name="qkvp", bufs=1))
        # PSUM: wide 2x4KB (ST/stats/broadcast) + lin 2x2KB + pv 2x2KB = 16KB
        ps_st = ctx.enter_context(tc.tile_pool(name="ps_st", bufs=2, space="PSUM"))
        ps_lin = ctx.enter_context(tc.tile_pool(name="ps_lin", bufs=2, space="PSUM"))
        ps_pv = ctx.enter_context(tc.tile_pool(name="ps_pv", bufs=2, space="PSUM"))

        ident_b = const.tile([P, P], BF16)
        ident_r = const.tile([P, P], F32R)
        identf = const.tile([P, P], F32)
        ones_f = const.tile([P, 1], F32)
        nc.vector.memset(ones_f[:], 1.0)
        ones8 = const.tile([P, NT], F32)
        nc.vector.memset(ones8[:], 1.0)
        ones_col_b = const.tile([P, 1], BF16)
        nc.scalar.activation(ones_col_b[:], ones_f[:], AF.Copy)
        ones_row = const.tile([1, P], F32R)
        eps_col = const.tile([P, 1], F32)
        nc.vector.memset(eps_col[:], EPS)
        zero_col = const.tile([P, 1], F32)
        nc.vector.memset(zero_col[:], 0.0)

        masks = []
        with tc.tile_pool(name="mbuild", bufs=2) as mbp:
            make_identity(nc, identf[:])
            nc.scalar.activation(ident_b[:], identf[:], AF.Copy)
            nc.scalar.activation(ident_r[:], identf[:], AF.Copy)
            ones_rowf = mbp.tile([1, P], F32, tag="orow", name="ones_rowf")
            nc.vector.memset(ones_rowf[:], 1.0)
            nc.scalar.activation(ones_row[:], ones_rowf[:], AF.Copy)
            # causal mask per diagonal d: mask_d[r, c] = 1 if c - r >= d*128
            for di in range(4):
                mf = mbp.tile([P, 512], F32, tag="mf", name=f"mf{di}")
                nc.gpsimd.memset(mf[:], 1.0)
                nc.gpsimd.affine_select(
                    out=mf[:], in_=mf[:], pattern=[[1, 512]],
                    base=-di * P, channel_multiplier=-1,
                    compare_op=ALU.is_ge, fill=0.0)
                mk = const.tile([P, 512], BF16, tag=f"mask{di}", name=f"mask{di}")
                nc.scalar.activation(mk[:], mf[:], AF.Copy)
                masks.append(mk)

        # bias columns (col m = vec[m*128:(m+1)*128]): DMA row-major here;
        # the PE transposes are deferred into the load loop so they don't
        # head-of-line-block the PE queue behind these late-arriving DMAs
        bqkv_t = const.tile([P, 3 * NCH], F32)
        qdq_t = const.tile([P, 3 * NCH], F32)
        bproj_t = const.tile([P, NCH], F32)
        b1_t = const.tile([P, FF // P], F32)
        b2_t = const.tile([P, NCH], F32)
        bias_jobs = []

        def emit_bias_dmas():
            for src_d, dst, nr in ((bqkv_d, bqkv_t, 3 * NCH),
                                   (qdq_d, qdq_t, 3 * NCH),
                                   (bproj_d, bproj_t, NCH),
                                   (b1_d, b1_t, FF // P),
                                   (b2_d, b2_t, NCH)):
                tmp = blp.tile([nr, P], F32, bufs=1,
                               tag=f"btmp{nr}_{len(bias_jobs)}",
                               name=f"btmp{len(bias_jobs)}")
                nc.sync.dma_start(tmp[:], src_d.rearrange("(m p) -> m p", p=P))
                bias_jobs.append((tmp, dst, nr))

        def emit_bias_transposes():
            for bi, (tmp, dst, nr) in enumerate(bias_jobs):
                pst = ps_lin.tile([P, 512], F32, tag="lin", name=f"btr{bi}")
                nc.tensor.transpose(pst[:, 0:nr], tmp[:], identf[0:nr, 0:nr])
                nc.scalar.activation(dst[:], pst[:, 0:nr], AF.Copy)

        # persistent feature-major big tiles: [P, chunk, T]
        xf = xpool.tile([P, NCH, T], BF16, tag="x", name="xf")
        h18 = hpool.tile([P, NCH, T], FP8, tag="h8", name="h18")  # h1 * SA

        def make_qkv(hb):
            """Alloc q/k/v + weight DMAs; emission via generator units.
            fp8 DoubleRow matmuls: each contracts a 256-row k-pair."""
            q = qkvp.tile([P, T], BF16, tag="qk", bufs=4, name=f"q{hb}")
            k = qkvp.tile([P, T], BF16, tag="qk", bufs=4, name=f"k{hb}")
            v = qkvp.tile([P, T], BF16, tag="v", bufs=2, name=f"v{hb}")
            wts = []
            for li, lname, dst in ((2, "v", v), (1, "k", k), (0, "q", q)):
                w = wpool.tile([P, NCH // 2, 2, P], FP8, tag="wqkv", bufs=3,
                               name=f"w{lname}{hb}")
                nc.sync.dma_start(w[:], Wqkv_d[li * NCH + hb])
                wts.append((li, lname, dst, w))

            def gen():
                for t in range(NQ):
                    for li, lname, dst, w in wts:
                        mcol = li * NCH + hb
                        ps = ps_lin.tile([P, 512], F32, tag="lin",
                                         name=f"{lname}{hb}ps{t}")
                        for jj in range(NCH // 2):
                            nc.tensor.matmul(
                                ps[:], w[:, jj],
                                h18[:, 2 * jj:2 * jj + 2, ts(t, 512)],
                                start=(jj == 0), stop=(jj == NCH // 2 - 1),
                                perf_mode=DR)
                            if jj < NCH // 2 - 1:
                                yield
                        if li == 0:
                            # q eviction on DVE; k/v on ACT to relieve the
                            # ~93%-busy DVE in the attention window
                            nc.vector.tensor_scalar(
                                dst[:, ts(t, 512)], ps[:],
                                qdq_t[:, mcol:mcol + 1],
                                bqkv_t[:, mcol:mcol + 1],
                                ALU.mult, ALU.add)
                        else:
                            nc.scalar.activation(
                                dst[:, ts(t, 512)], ps[:], AF.Identity,
                                bias=bqkv_t[:, mcol:mcol + 1],
                                scale=qdq_t[:, mcol:mcol + 1])
                        yield
            return q, k, v, gen()

        def tr_tile(dst_ap, src, nm, on_dve, evict_scale=None):
            """Transpose a [P, T] bf16 token-major tile into feature-major
            dst (a [P, 8, 128]-shaped AP), via one 2KB PSUM bank.  The
            eviction converts to dst dtype, optionally scaling."""
            pst = ps_lin.tile([P, NT, P], BF16, tag="lin", name=f"tr{nm}")
            for mi in range(NT):
                nc.tensor.transpose(pst[:, mi, :], src[:, ts(mi, P)],
                                    ident_b[:])
            if on_dve:
                if evict_scale is None:
                    nc.vector.tensor_copy(dst_ap, pst[:])
                else:
                    nc.vector.tensor_scalar_mul(dst_ap, pst[:], evict_scale)
            elif evict_scale is None:
                nc.scalar.activation(dst_ap, pst[:], AF.Copy)
            else:
                nc.scalar.activation(dst_ap, pst[:], AF.Identity,
                                     scale=evict_scale)

        def tr_tile_f32(dst_ap_g, src, nm):
            """Transpose a [P, T] f32r tile into feature-major bf16 dst via
            two [P,512] f32r PSUM groups; DVE eviction converts to bf16.
            dst_ap_g(g) -> [P, 4, 128]-shaped AP for chunk group g."""
            for g in range(2):
                pst = ps_lin.tile([P, 4, P], F32R, tag="lin",
                                  name=f"tr{nm}_{g}")
                for mi in range(4):
                    nc.tensor.transpose(pst[:, mi, :],
                                        src[:, ts(4 * g + mi, P)], ident_r[:])
                nc.vector.tensor_copy(dst_ap_g(g), pst[:])

        # ---------------- load x; token-major LN1 ---------------------------
        with tc.tile_pool(name="xload", bufs=8) as xlp:
            xtm = [xlp.tile([P, C], F32R, tag="xtm", bufs=8, name=f"xtm{i}")
                   for i in range(NT)]
            # htm[i] is consumed by tr_tile in the next iteration: 3 bufs
            htm = [xlp.tile([P, C], BF16, tag="htm", bufs=3, name=f"htm{i}")
                   for i in range(NT)]
            # NOTE: keep every DMA on the SP ring — splitting across the
            # Act ring produced NaNs on hardware (the scheduler's DMA
            # completion lanes assume single-ring FIFO order).  Tiny bias
            # DMAs + their transposes go first (PE warm-up at ~1us), then
            # x, then the QKV(0) weights: the shared completion-lane
            # counters make early consumers wait for everything emitted
            # before them on the lane.
            emit_bias_dmas()
            emit_bias_transposes()
            for i in range(NT):
                nc.sync.dma_start(xtm[i][:], x_d[ts(i, P), :])
            qkv0 = make_qkv(0)

            def ln1_tm(i):
                scr = xlp.tile([P, C], F32, tag="scr", bufs=1, name=f"scr{i}")
                cols = {}
                for nm in ("s", "q", "mu", "var", "musq", "sd", "inv", "c0"):
                    cols[nm] = spool.tile([P, 1], F32, tag="lncol", bufs=24,
                                          name=f"{nm}{i}")
                nc.vector.tensor_reduce(cols["s"][:], xtm[i][:], AX.X, ALU.add)
                nc.scalar.activation(scr[:], xtm[i][:], AF.Square,
                                     accum_out=cols["q"][:])
                nc.scalar.mul(cols["mu"][:], cols["s"][:], 1.0 / C)
                nc.scalar.mul(cols["var"][:], cols["q"][:], 1.0 / C)
                nc.vector.tensor_mul(cols["musq"][:], cols["mu"][:],
                                     cols["mu"][:])
                nc.vector.tensor_sub(cols["var"][:], cols["var"][:],
                                     cols["musq"][:])
                nc.scalar.activation(cols["sd"][:], cols["var"][:], AF.Sqrt,
                                     bias=eps_col[:])
                nc.vector.reciprocal(cols["inv"][:], cols["sd"][:])
                nc.vector.tensor_mul(cols["c0"][:], cols["mu"][:],
                                     cols["inv"][:])
                nc.scalar.mul(cols["c0"][:], cols["c0"][:], -1.0)
                nc.scalar.activation(htm[i][:], xtm[i][:], AF.Identity,
                                     bias=cols["c0"][:],
                                     scale=cols["inv"][:])

            def xtr(i):
                for g in range(2):
                    pst = ps_lin.tile([P, 4, P], F32R, tag="lin",
                                      name=f"trx{i}_{g}")
                    for mi in range(4):
                        nc.tensor.transpose(pst[:, mi, :],
                                            xtm[i][:, ts(4 * g + mi, P)],
                                            ident_r[:])
                    nc.vector.tensor_copy(xf[:, 4 * g:4 * g + 4, ts(i, P)],
                                          pst[:])

            for i in range(NT):
                ln1_tm(i)
                # xtr lags 2 tiles: its x-DMA landed ~4us ago, so it never
                # head-of-line blocks the h-chain in the in-order PE queue
                if i > 1:
                    xtr(i - 2)
                if i > 0:
                    tr_tile(h18[:, :, ts(i - 1, P)], htm[i - 1][:],
                            f"h{i - 1}", on_dve=(i % 2 == 0), evict_scale=SA)
                if i == 4:
                    # h1 rows for t=0 complete: overlap QKV(0) t=0 only
                    # (t=1 units would head-of-line block the in-order PE
                    # queue behind the not-yet-emitted transposes of 4..7)
                    for _ in range(12):
                        next(qkv0[3], None)
            tr_tile(h18[:, :, ts(NT - 1, P)], htm[NT - 1][:], f"h{NT - 1}",
                    on_dve=False, evict_scale=SA)
            xtr(NT - 2)
            xtr(NT - 1)

        # ---------------- attention: pipelined per head-block ----------------
        with tc.tile_pool(name="ptp", bufs=1) as ptp, \
             tc.tile_pool(name="ypool", bufs=1) as ypool:

            yf = ypool.tile([P, NCH, T], BF16, tag="y", name="yf")

            def v_transpose(hb, v):
                va = qkvp.tile([P, NT, 130], BF16, tag="vaug", bufs=2,
                               name=f"va{hb}")
                pst = ps_lin.tile([P, NT, P], BF16, tag="lin", name=f"vtr{hb}")
                for ki in range(NT):
                    nc.tensor.transpose(pst[:, ki, :], v[:, ts(ki, P)],
                                        ident_b[:])
                dst = va[:, :, 0:130].rearrange(
                    "p k (h c) -> p k h c", h=2)[:, :, :, 0:64]
                src = pst[:].rearrange("p k (h c) -> p k h c", h=2)
                nc.vector.tensor_copy(dst, src)
                nc.vector.tensor_copy(
                    va[:, :, 64:65].rearrange("p k o -> p (k o)"), ones8[:])
                nc.vector.tensor_copy(
                    va[:, :, 129:130].rearrange("p k o -> p (k o)"), ones8[:])
                return va

            def emit_st(hb, qi, ki, q, k):
                """One wide ST tile + exp (+mask); returns the P tile."""
                st = ps_st.tile([P, T], F32, tag="st", name=f"st{hb}_{qi}_{ki}")
                for p_ in range(2):
                    nc.tensor.matmul(
                        st[:, ts(p_, 512)],
                        k[p_ * 64:(p_ + 1) * 64, ts(ki, P)],
                        q[p_ * 64:(p_ + 1) * 64, ts(qi, 512)],
                        start=True, stop=True)
                pt = ptp.tile([P, T], BF16, tag="pt", bufs=12,
                              name=f"pt{hb}_{qi}_{ki}")
                nc.scalar.activation(pt[:], st[:], AF.Exp,
                                     bias=zero_col[:], scale=SCALE)
                d = ki - 4 * qi
                if d >= 0:
                    # NOTE: keep these on DVE — gpsimd adds ~1.2us semaphore
                    # latency per cross-engine handoff and stalls PV
                    for p_ in range(2):
                        nc.vector.tensor_mul(pt[:, ts(p_, 512)],
                                             pt[:, ts(p_, 512)], masks[d][:])
                return pt

            def make_pv(hb, qi, va, pts, out):
                """Generator: PV accumulation in 2-mm units, then dn chain."""
                def gen():
                    pvs = []
                    kmax = 4 * qi + 3
                    for p_ in range(2):
                        pv = ps_pv.tile([P, 512], F32, tag="pv",
                                        name=f"pv{hb}_{qi}_{p_}")
                        for ki in range(kmax + 1):
                            nc.tensor.matmul(
                                pv[0:65, :],
                                va[:, ki, p_ * 65:(p_ + 1) * 65],
                                pts[ki][:, ts(p_, 512)],
                                start=(ki == 0), stop=(ki == kmax))
                            if ki % 2 == 1:
                                yield
                        pvs.append(pv)
                    # raw denominators in f32r; the reciprocal runs wide
                    # (approx_fast — the accurate wide reciprocal is a
                    # ~6.5us multi-pass Newton sequence, measured) on the
                    # [64,T] broadcast in dn_bcast_finish
                    dnr = qkvp.tile([1, T], F32R, tag="dnrow", bufs=3,
                                    name=f"dr{hb}_{qi}")
                    for p_ in range(2):
                        nc.vector.tensor_copy(dnr[0:1, ts(p_, 512)],
                                              pvs[p_][64:65, :])
                    out.extend([pvs, dnr])
                return gen()

            def dn_bcast_finish(hb, qi, pvs, dnr, use_lin=False):
                """Broadcast raw denominators over 64 partitions, take the
                reciprocal wide, write y (bf16)."""
                dnb = qkvp.tile([64, T], F32, tag="dnb", bufs=1,
                                name=f"dnbs{hb}_{qi}")
                if use_lin:
                    for p_ in range(2):
                        bps = ps_lin.tile([P, 512], F32, tag="lin",
                                          name=f"dnb{hb}_{qi}_{p_}")
                        nc.tensor.matmul(bps[0:64, :], ones_row[:, 0:64],
                                         dnr[0:1, ts(p_, 512)],
                                         start=True, stop=True)
                        nc.vector.reciprocal_approx_fast(dnb[:, ts(p_, 512)],
                                                         bps[0:64, :])
                else:
                    bps = ps_st.tile([P, T], F32, tag="st",
                                     name=f"dnb{hb}_{qi}")
                    for p_ in range(2):
                        nc.tensor.matmul(bps[0:64, ts(p_, 512)],
                                         ones_row[:, 0:64],
                                         dnr[0:1, ts(p_, 512)],
                                         start=True, stop=True)
                    nc.vector.reciprocal_approx_fast(dnb[:], bps[0:64, :])
                for p_ in range(2):
                    nc.vector.tensor_mul(
                        yf[p_ * 64:(p_ + 1) * 64, hb, ts(qi, 512)],
                        pvs[p_][0:64, :], dnb[:, ts(p_, 512)])

            def pump(g, n=None):
                if g is None:
                    return True
                try:
                    if n is None:
                        while True:
                            next(g)
                    else:
                        for _ in range(n):
                            next(g)
                except StopIteration:
                    return True
                return False

            # --- pipelined head-block loop ---
            q, k, v, gq = qkv0
            pump(gq)                       # finish QKV(0) (partly ran in load)
            gq = None
            prev1 = None                   # (hb, va, pts1) -> PV in next iter
            pend0 = None                   # (hb, pvs0, dnr0) -> bcast next iter
            for hb in range(NCH):
                va = v_transpose(hb, v)
                if hb < NCH - 1:
                    qn, kn, vn, gq = make_qkv(hb + 1)
                else:
                    qn = kn = vn = gq = None
                res1 = []
                gpv1 = None
                pv1_done = prev1 is None
                if prev1 is not None:
                    phb, pva, ppts1 = prev1
                    gpv1 = make_pv(phb, 1, pva, ppts1, res1)
                res0 = []
                gpv0 = None
                pts0, pts1 = [], []
                st_items = [(0, ki) for ki in range(4)] + \
                           [(1, ki) for ki in range(NT)]
                for idx, (qi, ki) in enumerate(st_items):
                    pt = emit_st(hb, qi, ki, q, k)
                    (pts0 if qi == 0 else pts1).append(pt)
                    if idx == 0 and pend0 is not None:
                        dn_bcast_finish(pend0[0], 0, pend0[1], pend0[2])
                        pend0 = None
                    if gpv1 is None and not pv1_done:
                        dn_bcast_finish(phb, 1, res1[0], res1[1])
                        pv1_done = True
                    if idx == 7:
                        if gpv1 is not None:
                            pump(gpv1)
                            gpv1 = None
                        if not pv1_done:
                            dn_bcast_finish(phb, 1, res1[0], res1[1])
                            pv1_done = True
                        gpv0 = make_pv(hb, 0, va, pts0, res0)
                    for _ in range(3):
                        if gpv1 is not None:
                            if pump(gpv1, 1):
                                gpv1 = None
                        elif idx >= 8 and gpv0 is not None:
                            if pump(gpv0, 1):
                                gpv0 = None
                        elif gq is not None:
                            if pump(gq, 1):
                                gq = None
                pump(gpv0)
                pump(gq)
                gq = None
                pend0 = (hb, res0[0], res0[1])
                prev1 = (hb, va, pts1)
                q, k, v = qn, kn, vn
                if hb == NCH - 1:
                    # last head-block: finish (7, q0) now instead of in the
                    # serial leftover section
                    dn_bcast_finish(pend0[0], 0, pend0[1], pend0[2])
                    pend0 = None

            # prefetch proj + FFN1-qtr0 weights before the leftovers
            ws = {}
            for m in range(2):
                w = wpool.tile([P, NCH, P], BF16, tag="wqkv", bufs=3,
                               name=f"wproj{m}_pre")
                nc.sync.dma_start(w[:], Wproj_d[m])
                ws[m] = w
            w1_pre, a1_pre = [], []
            for mm_ in range(8):
                wt = ffnp.tile([P, NCH, P], BF16, tag="w1", bufs=8,
                               name=f"w1_{mm_}")
                nc.sync.dma_start(wt[:], W1_d[mm_])
                a = ffnp.tile([P, T], BF16, tag="a1", bufs=8, name=f"a1_{mm_}")
                w1_pre.append(wt)
                a1_pre.append(a)
            w2_pre = {}
            for m in range(2):
                w2t = ffnp.tile([P, 8, P], BF16, tag="w2", bufs=3,
                                name=f"w2_0_{m}")
                nc.sync.dma_start(w2t[:], W2_d[m])
                w2_pre[m] = w2t

            # --- leftovers + proj + LN2 ---
            if pend0 is not None:
                dn_bcast_finish(pend0[0], 0, pend0[1], pend0[2])
            phb, pva, ppts1 = prev1
            res1 = []
            pump(make_pv(phb, 1, pva, ppts1, res1))
            # bproj pre-add into the residual (ACT; r1 = x + bproj + y@Wp)
            for m in range(NCH):
                nc.scalar.activation(xf[:, m, :], xf[:, m, :], AF.Identity,
                                     bias=bproj_t[:, m:m + 1], scale=1.0)

            ln2_stats = [ps_st.tile([1, T], F32, tag="st", name=f"ln2_st{t}")
                         for t in range(NQ)]

            def ln2_sq(m, t):
                sq = spool.tile([P, 512], BF16, tag="sq", bufs=2,
                                name=f"ln2sq{m}_{t}")
                nc.gpsimd.tensor_mul(sq[:], xf[:, m, ts(t, 512)],
                                     xf[:, m, ts(t, 512)])
                return sq

            def ln2_stat_mms(m, t, sq):
                nc.tensor.matmul(ln2_stats[t][0:1, 0:512],
                                 ones_col_b[:], xf[:, m, ts(t, 512)],
                                 start=(m == 0), stop=(m == NCH - 1))
                nc.tensor.matmul(ln2_stats[t][0:1, 512:1024],
                                 ones_col_b[:], sq[:],
                                 start=(m == 0), stop=(m == NCH - 1))

            def ln2_finalize(t):
                # broadcast the RAW stats across partitions first (PE), then
                # do all the math as wide [128,512] ops at full engine rate —
                # single-partition [1,512] DVE ops run one lane (~2.4us each)
                srow = spool.tile([1, T], F32R, tag="lnrows", bufs=1,
                                  name=f"ln2srow{t}")
                nc.scalar.activation(srow[:], ln2_stats[t][0:1, :], AF.Copy)
                bps = ps_st.tile([P, T], F32, tag="st", name=f"ln2bps{t}")
                for half in range(2):
                    nc.tensor.matmul(bps[:, ts(half, 512)], ones_row[:],
                                     srow[0:1, ts(half, 512)],
                                     start=True, stop=True)
                wt = lambda nm: spool.tile([P, 512], F32, tag="lnw", bufs=4,
                                           name=f"ln2{nm}{t}")
                mu_t, msq_t, var_t, c0f = wt("mu"), wt("msq"), wt("var"), \
                    wt("c0f")
                nc.scalar.mul(mu_t[:], bps[:, 0:512], 1.0 / C)
                nc.scalar.activation(msq_t[:], mu_t[:], AF.Square)
                nc.scalar.mul(var_t[:], bps[:, 512:1024], 1.0 / C)
                nc.vector.tensor_sub(var_t[:], var_t[:], msq_t[:])
                nc.scalar.activation(var_t[:], var_t[:], AF.Sqrt,
                                     bias=eps_col[:])
                # bf16 broadcast rows: the apply's mul runs at 2x DVE rate
                bc = spool.tile([P, T], BF16, tag="lnbc", bufs=2,
                                name=f"ln2bc{t}")
                nc.vector.reciprocal(bc[:, 0:512], var_t[:])
                nc.vector.tensor_mul(c0f[:], mu_t[:], bc[:, 0:512])
                nc.scalar.mul(bc[:, 512:1024], c0f[:], -1.0)
                return bc

            h2f = hpool.tile([P, NCH, T], BF16, tag="h", name="h2f")

            def ln2_apply(c, t, bc):
                # DVE staged via PSUM (3-SBUF-operand DVE ops run at 1/3
                # rate; gpsimd is far too slow for bulk elementwise work)
                ps = ps_pv.tile([P, 512], F32, tag="pv", name=f"ap{c}_{t}")
                nc.vector.tensor_mul(ps[:], xf[:, c, ts(t, 512)],
                                     bc[:, 0:512])
                nc.vector.tensor_add(h2f[:, c, ts(t, 512)], ps[:],
                                     bc[:, 512:1024])

            def proj_pass(t, first, bc_prev=None):
                def load(m):
                    if m < NCH and m not in ws:
                        w = wpool.tile([P, NCH, P], BF16, tag="wqkv", bufs=3,
                                       name=f"wproj{m}_{t}")
                        nc.sync.dma_start(w[:], Wproj_d[m])
                        ws[m] = w

                load(0)
                load(1)
                sqs = {}
                for m in range(NCH):
                    load(m + 2)
                    ps = ps_lin.tile([P, 512], F32, tag="lin",
                                     name=f"proj_ps{m}_{t}")
                    for j in range(NCH):
                        nc.tensor.matmul(ps[:], ws[m][:, j, :],
                                         yf[:, j, ts(t, 512)],
                                         start=(j == 0), stop=(j == NCH - 1))
                    if first and m == 0:
                        # finish (7, q1) while proj keeps the PE busy
                        dn_bcast_finish(phb, 1, res1[0], res1[1], use_lin=True)
                    nc.vector.tensor_add(xf[:, m, ts(t, 512)],
                                         xf[:, m, ts(t, 512)], ps[:])
                    # squares on gpsimd; stat matmuls lag one group so the PE
                    # never waits on the gpsimd queue
                    sqs[m] = ln2_sq(m, t)
                    if m > 0:
                        ln2_stat_mms(m - 1, t, sqs[m - 1])
                    if bc_prev is not None:
                        # interleave prev-t LN2 applies between this pass's
                        # DVE adds so neither chain delays the other's
                        # downstream consumers (stats t / FFN1 prev-t)
                        ln2_apply(m, 1 - t, bc_prev)
                ln2_stat_mms(NCH - 1, t, sqs[NCH - 1])
                if t == 0:
                    ws.clear()   # re-DMA per t (tiles recycled, bufs=3)

            proj_pass(0, True)
            bc0 = ln2_finalize(0)
            proj_pass(1, False, bc_prev=bc0)
            # finalize(1) / apply(1) / b2 pre-add are emitted inside the
            # FFN section, after the first FFN1 t=0 groups, so their
            # latency hides under PE matmul work

        # ---------------- FFN (4 d_ff quarters) + residual + out -------------
        qkv_ctx.close()
        if True:
            def ffn1_group(mg, t, a, wt):
                ps = ps_lin.tile([P, 512], F32, tag="lin",
                                 name=f"f1ps{mg}_{t}")
                for j in range(NCH):
                    nc.tensor.matmul(ps[:], wt[:, j, :],
                                     h2f[:, j, ts(t, 512)],
                                     start=(j == 0), stop=(j == NCH - 1))
                nc.scalar.activation(a[:, ts(t, 512)], ps[:], AF.Relu,
                                     bias=b1_t[:, mg:mg + 1], scale=1.0)

            w1_tiles = {0: w1_pre}
            for qtr in range(4):
                wts = w1_tiles.pop(qtr)
                if qtr == 0:
                    a1 = a1_pre
                else:
                    a1 = [ffnp.tile([P, T], BF16, tag="a1", bufs=8,
                                    name=f"a1_{qtr * 8 + mm_}")
                          for mm_ in range(8)]
                for mm_ in range(8):
                    ffn1_group(qtr * 8 + mm_, 0, a1[mm_], wts[mm_])
                if qtr == 0:
                    bc1 = ln2_finalize(1)
                    for c in range(NCH):
                        ln2_apply(c, 1, bc1)   # DVE; overlaps FFN1 t=0
                    for m in range(NCH):       # pre-add b2 (stats read done)
                        nc.scalar.activation(xf[:, m, :], xf[:, m, :],
                                             AF.Identity,
                                             bias=b2_t[:, m:m + 1], scale=1.0)
                for mm_ in range(8):
                    ffn1_group(qtr * 8 + mm_, 1, a1[mm_], wts[mm_])
                if qtr + 1 < 4:
                    # issue next quarter's W1 DMAs now: the tile buffers'
                    # readers (this quarter's FFN1) are already emitted, so
                    # the DMAs stream in during FFN2
                    nxt = []
                    for mm_ in range(8):
                        mg = (qtr + 1) * 8 + mm_
                        wt = ffnp.tile([P, NCH, P], BF16, tag="w1", bufs=8,
                                       name=f"w1_{mg}")
                        nc.sync.dma_start(wt[:], W1_d[mg])
                        nxt.append(wt)
                    w1_tiles[qtr + 1] = nxt

                w2s = w2_pre if qtr == 0 else {}

                def load2(m, qtr=qtr, w2s=w2s):
                    if m < NCH and m not in w2s:
                        w2t = ffnp.tile([P, 8, P], BF16, tag="w2", bufs=3,
                                        name=f"w2_{qtr}_{m}")
                        nc.sync.dma_start(w2t[:], W2_d[qtr * 8 + m])
                        w2s[m] = w2t

                load2(0)
                load2(1)
                for m in range(NCH):
                    load2(m + 2)
                    for t in range(NQ):
                        ps = ps_lin.tile([P, 512], F32, tag="lin",
                                         name=f"f2ps{qtr}_{m}_{t}")
                        for j in range(8):
                            nc.tensor.matmul(ps[:], w2s[m][:, j, :],
                                             a1[j][:, ts(t, 512)],
                                             start=(j == 0), stop=(j == 7))
                        nc.vector.tensor_add(xf[:, m, ts(t, 512)],
                                             xf[:, m, ts(t, 512)], ps[:])
                    if qtr == 3:
                        # xf[:, m] final: per-half transpose + evict + DMA
                        # so the last output DMA starts as early as possible
                        om = ffnp.tile([P, T], F32, tag="om", bufs=2,
                                       name=f"om{m}")
                        omr = om[:].rearrange("p (g i f) -> p g i f",
                                              g=2, i=4)
                        dst = out_d[:, ts(m, P)].rearrange(
                            "(g i p) f -> g p i f", g=2, p=P)
                        pst = ps_lin.tile([P, NT, P], BF16, tag="lin",
                                          name=f"otr{m}")
                        for g in range(2):
                            for ii in range(4):
                                nc.tensor.transpose(
                                    pst[:, 4 * g + ii, :],
                                    xf[:, m, ts(4 * g + ii, P)], ident_b[:])
                            if m % 2 == 0:
                                nc.scalar.activation(
                                    omr[:, g], pst[:, 4 * g:4 * g + 4, :],
                                    AF.Copy)
                            else:
                                nc.vector.tensor_copy(
                                    omr[:, g], pst[:, 4 * g:4 * g + 4, :])
                            nc.sync.dma_start(dst[g], omr[:, g])

    nc.compile()
    return nc


_NC_CACHE = {}


def _get_nc():
    if "nc" not in _NC_CACHE:
        _NC_CACHE["nc"] = _build()
    return _NC_CACHE["nc"]


def _fold_inputs(inputs):
    """Fold LN gains/biases into downstream weights; cast bf16; repack
    tile-major so each [128, kchunks, 128] weight tile is one contiguous
    DMA."""
    import ml_dtypes

    f = lambda kk: np.asarray(inputs[kk], dtype=np.float32)
    Wqkv, bqkv = f("Wqkv"), f("bqkv")
    W1, b1 = f("W1"), f("b1")
    ln1_g, ln1_b = f("ln1_g"), f("ln1_b")
    ln2_g, ln2_b = f("ln2_g"), f("ln2_b")

    def pack(w):
        # [K, M] -> [M/128 tiles, 128 kpart, K/128 kchunk, 128 mcol]
        K, M = w.shape
        t = w.reshape(K // P, P, M // P, P).transpose(2, 1, 0, 3)
        return np.ascontiguousarray(t.astype(ml_dtypes.bfloat16))

    def pack_fp8_dr(w, sa):
        # per-output-chunk e4m3 quantization + DoubleRow k-pair layout:
        # [K, M] -> [M/128 tiles, 128 kpart, K/256 pairs, 2, 128 mcol];
        # returns (tiles, dequant vector [M] = 1/(sa*sw_chunk))
        K, M = w.shape
        nt = M // P
        sw = 240.0 / np.abs(w.reshape(K, nt, P)).max(axis=(0, 2))  # [nt]
        w8 = np.clip(w.reshape(K, nt, P) * sw[None, :, None],
                     -240, 240).astype(ml_dtypes.float8_e4m3fn)
        t = w8.reshape(K // 256, 2, P, nt, P).transpose(3, 2, 0, 1, 4)
        dq = np.repeat(1.0 / (sa * sw), P).astype(np.float32)
        return np.ascontiguousarray(t), np.ascontiguousarray(dq)

    def pack_w2(w):
        # [4096, 1024] -> [(qtr m) tiles, 128, 8 kchunk-in-qtr, 128]
        K, M = w.shape
        t = w.reshape(4, 8, P, M // P, P)          # qtr, j, p, m, c
        t = t.transpose(0, 3, 2, 1, 4).reshape(4 * (M // P), P, 8, P)
        return np.ascontiguousarray(t.astype(ml_dtypes.bfloat16))

    Wq8, qdq = pack_fp8_dr(ln1_g[:, None] * Wqkv, 16.0)
    return {
        "Wqkv": Wq8,
        "qdq": qdq,
        "bqkv": np.ascontiguousarray(bqkv + ln1_b @ Wqkv),
        "Wproj": pack(f("Wproj")),
        "bproj": np.ascontiguousarray(f("bproj")),
        "W1": pack(ln2_g[:, None] * W1),
        "b1": np.ascontiguousarray(b1 + ln2_b @ W1),
        "W2": pack_w2(f("W2")),
        "b2": np.ascontiguousarray(f("b2")),
    }


def kernel(**inputs):
    from concourse.bass_utils import run_bass_kernel_spmd

    nc = _get_nc()
    shared = _fold_inputs(inputs)
    x = np.asarray(inputs["x"], dtype=np.float32)
    in_maps = [dict(shared, x=np.ascontiguousarray(x[i])) for i in range(B)]
    res = run_bass_kernel_spmd(nc, in_maps, core_ids=list(range(B)))
    out = np.stack([res.results[i]["out"] for i in range(B)], axis=0)
    return out.astype(np.float32)


# revision 76
# speedup vs baseline: 1.0030x; 1.0030x over previous
"""Trainium2 Bass kernel for one GPT-style transformer block.

Problem: x[8,1024,1024]; per-core = one batch element (data-parallel over 8
NeuronCores).  Per core:
    h1 = LN(x); qkv = h1@Wqkv+b; causal MHA (16 heads, d=64);
    r1 = x + attn@Wproj+b; h2 = LN(r1); out = r1 + relu(h2@W1+b1)@W2+b2

v2 (all-bf16): v1 measured 600 us/core with PE busy 517 us; 1800 of 2248
matmuls ran with fp32 weights (LDWEIGHTS 187 ns vs 97 bf16; issue spacing
234 vs 215 ns) and 196 fp32 PE transposes at ~270 ns.  v2 moves every
matmul operand to bf16 (numpy sim: rel err 3.5e-3 vs 2e-2 budget):
  - weights folded (LN gains into Wqkv/W1) then cast bf16 and repacked
    tile-major on the host so each weight tile is one contiguous DMA.
  - activations h1/y/h2/a1 and the residual xf all bf16; LN stats and
    PSUM accumulation stay f32.
  - all PE transposes bf16 (x is cast to bf16 on gpsimd first); each
    128x1024 tile transposes through one 2KB PSUM bank (8 mms + 1 evict).
  - x feature-major transposes are deferred into the attention loop as
    PE filler; LN1 applies split ACT/DVE; casts on gpsimd.
  - bproj/b2 residual pre-adds on gpsimd; LN2 apply split DVE (t=0) /
    gpsimd (t=1); Wproj/W1-qtr0 prefetched during attention.
  - FFN2 out-path: 8 transposes + 1 wide evict per chunk, out-DMA split
    per 512-token half.
"""

import math
import sys

import numpy as np

sys.path.insert(0, "/opt/trn_rl_repo")

from contextlib import ExitStack

import concourse.bass as bass
import concourse.mybir as mybir
import concourse.tile as tile
from concourse import bacc
from concourse.bass import ts
from concourse.masks import make_identity

F32 = mybir.dt.float32
F32R = mybir.dt.float32r
BF16 = mybir.dt.bfloat16
FP8 = mybir.dt.float8e4
DR = mybir.MatmulPerfMode.DoubleRow
SA = 16.0           # fixed activation quant scale for h1 (LN output, |h|<~7)
AF = mybir.ActivationFunctionType
ALU = mybir.AluOpType
AX = mybir.AxisListType

B, T, C, H = 8, 1024, 1024, 16
D = C // H
FF = 4 * C
P = 128
NCH = C // P          # 8 feature chunks
NT = T // P           # 8 token chunks of 128
NQ = T // 512         # 2 token chunks of 512
SCALE = 1.0 / math.sqrt(3 * C // H)
EPS = 1e-5


def _build():
    nc = bacc.Bacc("TRN2", target_bir_lowering=False, debug=False)

    x_d = nc.dram_tensor("x", [T, C], F32R, kind="ExternalInput").ap()
    # weights are host-repacked tile-major: tile ti = [128 kpart, kchunks, 128]
    # Wqkv is fp8e4 with DoubleRow k-pair layout [128, 4 pairs, 2, 128]
    Wqkv_d = nc.dram_tensor("Wqkv", [3 * NCH, P, NCH // 2, 2, P], FP8,
                            kind="ExternalInput").ap()
    bqkv_d = nc.dram_tensor("bqkv", [3 * C], F32, kind="ExternalInput").ap()
    qdq_d = nc.dram_tensor("qdq", [3 * C], F32, kind="ExternalInput").ap()
    Wproj_d = nc.dram_tensor("Wproj", [NCH, P, NCH, P], BF16,
                             kind="ExternalInput").ap()
    bproj_d = nc.dram_tensor("bproj", [C], F32, kind="ExternalInput").ap()
    W1_d = nc.dram_tensor("W1", [FF // P, P, NCH, P], BF16,
                          kind="ExternalInput").ap()
    b1_d = nc.dram_tensor("b1", [FF], F32, kind="ExternalInput").ap()
    W2_d = nc.dram_tensor("W2", [FF // P, P, 8, P], BF16,
                          kind="ExternalInput").ap()
    b2_d = nc.dram_tensor("b2", [C], F32, kind="ExternalInput").ap()
    out_d = nc.dram_tensor("out", [T, C], F32, kind="ExternalOutput").ap()

    with nc.allow_low_precision(reason="bf16 matmul inputs (fp32 accum)"), \
         tile.TileContext(nc) as tc, ExitStack() as ctx:
        const = ctx.enter_context(tc.tile_pool(name="const", bufs=1))
        xpool = ctx.enter_context(tc.tile_pool(name="xpool", bufs=1))
        hpool = ctx.enter_context(tc.tile_pool(name="hpool", bufs=1))
        spool = ctx.enter_context(tc.tile_pool(name="spool", bufs=2))
        wpool = ctx.enter_context(tc.tile_pool(name="wpool", bufs=2))
        ffnp = ctx.enter_context(tc.tile_pool(name="ffnp", bufs=1))
        blp = ctx.enter_context(tc.tile_pool(name="bload", bufs=1))
        qkv_ctx = ExitStack()   # closed just before the FFN block
        qkvp = qkv_ctx.enter_context(tc.tile_pool(name="qkvp", bufs=1))
        # PSUM: wide 2x4KB (ST/stats/broadcast) + lin 2x2KB + pv 2x2KB = 16KB
        ps_st = ctx.enter_context(tc.tile_pool(name="ps_st", bufs=2, space="PSUM"))
        ps_lin = ctx.enter_context(tc.tile_pool(name="ps_lin", bufs=2, space="PSUM"))
        ps_pv = ctx.enter_context(tc.tile_pool(name="ps_pv", bufs=2, space="PSUM"))

        ident_b = const.tile([P, P], BF16)
        ident_r = const.tile([P, P], F32R)
        identf = const.tile([P, P], F32)
        ones_f = const.tile([P, 1], F32)
        nc.vector.memset(ones_f[:], 1.0)
        ones8 = const.tile([P, NT], F32)
        nc.vector.memset(ones8[:], 1.0)
        ones_col_b = const.tile([P, 1], BF16)
        nc.scalar.activation(ones_col_b[:], ones_f[:], AF.Copy)
        ones_row = const.tile([1, P], F32R)
        eps_col = const.tile([P, 1], F32)
        nc.vector.memset(eps_col[:], EPS)
        zero_col = const.tile([P, 1], F32)
        nc.vector.memset(zero_col[:], 0.0)

        masks = []
        with tc.tile_pool(name="mbuild", bufs=2) as mbp:
            make_identity(nc, identf[:])
            nc.scalar.activation(ident_b[:], identf[:], AF.Copy)
            nc.scalar.activation(ident_r[:], identf[:], AF.Copy)
            ones_rowf = mbp.tile([1, P], F32, tag="orow", name="ones_rowf")
            nc.vector.memset(ones_rowf[:], 1.0)
            nc.scalar.activation(ones_row[:], ones_rowf[:], AF.Copy)
            # causal mask per diagonal d: mask_d[r, c] = 1 if c - r >= d*128
            for di in range(4):
                mf = mbp.tile([P, 512], F32, tag="mf", name=f"mf{di}")
                nc.gpsimd.memset(mf[:], 1.0)
                nc.gpsimd.affine_select(
                    out=mf[:], in_=mf[:], pattern=[[1, 512]],
                    base=-di * P, channel_multiplier=-1,
                    compare_op=ALU.is_ge, fill=0.0)
                mk = const.tile([P, 512], BF16, tag=f"mask{di}", name=f"mask{di}")
                nc.scalar.activation(mk[:], mf[:], AF.Copy)
                masks.append(mk)

        # bias columns (col m = vec[m*128:(m+1)*128]): DMA row-major here;
        # the PE transposes are deferred into the load loop so they don't
        # head-of-line-block the PE queue behind these late-arriving DMAs
        bqkv_t = const.tile([P, 3 * NCH], F32)
        qdq_t = const.tile([P, 3 * NCH], F32)
        bproj_t = const.tile([P, NCH], F32)
        b1_t = const.tile([P, FF // P], F32)
        b2_t = const.tile([P, NCH], F32)
        bias_jobs = []

        def emit_bias_dmas():
            for src_d, dst, nr in ((bqkv_d, bqkv_t, 3 * NCH),
                                   (qdq_d, qdq_t, 3 * NCH),
                                   (bproj_d, bproj_t, NCH),
                                   (b1_d, b1_t, FF // P),
                                   (b2_d, b2_t, NCH)):
                tmp = blp.tile([nr, P], F32, bufs=1,
                               tag=f"btmp{nr}_{len(bias_jobs)}",
                               name=f"btmp{len(bias_jobs)}")
                nc.sync.dma_start(tmp[:], src_d.rearrange("(m p) -> m p", p=P))
                bias_jobs.append((tmp, dst, nr))

        def emit_bias_transposes():
            for bi, (tmp, dst, nr) in enumerate(bias_jobs):
                pst = ps_lin.tile([P, 512], F32, tag="lin", name=f"btr{bi}")
                nc.tensor.transpose(pst[:, 0:nr], tmp[:], identf[0:nr, 0:nr])
                nc.scalar.activation(dst[:], pst[:, 0:nr], AF.Copy)

        # persistent feature-major big tiles: [P, chunk, T]
        xf = xpool.tile([P, NCH, T], BF16, tag="x", name="xf")
        h18 = hpool.tile([P, NCH, T], FP8, tag="h8", name="h18")  # h1 * SA

        def make_qkv(hb):
            """Alloc q/k/v + weight DMAs; emission via generator units.
            fp8 DoubleRow matmuls: each contracts a 256-row k-pair."""
            q = qkvp.tile([P, T], BF16, tag="qk", bufs=4, name=f"q{hb}")
            k = qkvp.tile([P, T], BF16, tag="qk", bufs=4, name=f"k{hb}")
            v = qkvp.tile([P, T], BF16, tag="v", bufs=2, name=f"v{hb}")
            wts = []
            for li, lname, dst in ((2, "v", v), (1, "k", k), (0, "q", q)):
                w = wpool.tile([P, NCH // 2, 2, P], FP8, tag="wqkv", bufs=3,
                               name=f"w{lname}{hb}")
                nc.sync.dma_start(w[:], Wqkv_d[li * NCH + hb])
                wts.append((li, lname, dst, w))

            def gen():
                for t in range(NQ):
                    for li, lname, dst, w in wts:
                        mcol = li * NCH + hb
                        ps = ps_lin.tile([P, 512], F32, tag="lin",
                                         name=f"{lname}{hb}ps{t}")
                        for jj in range(NCH // 2):
                            nc.tensor.matmul(
                                ps[:], w[:, jj],
                                h18[:, 2 * jj:2 * jj + 2, ts(t, 512)],
                                start=(jj == 0), stop=(jj == NCH // 2 - 1),
                                perf_mode=DR)
                            if jj < NCH // 2 - 1:
                                yield
                        if li == 0:
                            # q eviction on DVE; k/v on ACT to relieve the
                            # ~93%-busy DVE in the attention window
                            nc.vector.tensor_scalar(
                                dst[:, ts(t, 512)], ps[:],
                                qdq_t[:, mcol:mcol + 1],
                                bqkv_t[:, mcol:mcol + 1],
                                ALU.mult, ALU.add)
                        else:
                            nc.scalar.activation(
                                dst[:, ts(t, 512)], ps[:], AF.Identity,
                                bias=bqkv_t[:, mcol:mcol + 1],
                                scale=qdq_t[:, mcol:mcol + 1])
                        yield
            return q, k, v, gen()

        def tr_tile(dst_ap, src, nm, on_dve, evict_scale=None):
            """Transpose a [P, T] bf16 token-major tile into feature-major
            dst (a [P, 8, 128]-shaped AP), via one 2KB PSUM bank.  The
            eviction converts to dst dtype, optionally scaling."""
            pst = ps_lin.tile([P, NT, P], BF16, tag="lin", name=f"tr{nm}")
            for mi in range(NT):
                nc.tensor.transpose(pst[:, mi, :], src[:, ts(mi, P)],
                                    ident_b[:])
            if on_dve:
                if evict_scale is None:
                    nc.vector.tensor_copy(dst_ap, pst[:])
                else:
                    nc.vector.tensor_scalar_mul(dst_ap, pst[:], evict_scale)
            elif evict_scale is None:
                nc.scalar.activation(dst_ap, pst[:], AF.Copy)
            else:
                nc.scalar.activation(dst_ap, pst[:], AF.Identity,
                                     scale=evict_scale)

        def tr_tile_f32(dst_ap_g, src, nm):
            """Transpose a [P, T] f32r tile into feature-major bf16 dst via
            two [P,512] f32r PSUM groups; DVE eviction converts to bf16.
            dst_ap_g(g) -> [P, 4, 128]-shaped AP for chunk group g."""
            for g in range(2):
                pst = ps_lin.tile([P, 4, P], F32R, tag="lin",
                                  name=f"tr{nm}_{g}")
                for mi in range(4):
                    nc.tensor.transpose(pst[:, mi, :],
                                        src[:, ts(4 * g + mi, P)], ident_r[:])
                nc.vector.tensor_copy(dst_ap_g(g), pst[:])

        # ---------------- load x; token-major LN1 ---------------------------
        with tc.tile_pool(name="xload", bufs=8) as xlp:
            xtm = [xlp.tile([P, C], F32R, tag="xtm", bufs=8, name=f"xtm{i}")
                   for i in range(NT)]
            # htm[i] is consumed by tr_tile in the next iteration: 3 bufs
            htm = [xlp.tile([P, C], BF16, tag="htm", bufs=3, name=f"htm{i}")
                   for i in range(NT)]
            # NOTE: keep every DMA on the SP ring — splitting across the
            # Act ring produced NaNs on hardware (the scheduler's DMA
            # completion lanes assume single-ring FIFO order).  Tiny bias
            # DMAs + their transposes go first (PE warm-up at ~1us), then
            # x, then the QKV(0) weights: the shared completion-lane
            # counters make early consumers wait for everything emitted
            # before them on the lane.
            emit_bias_dmas()
            emit_bias_transposes()
            for i in range(NT):
                nc.sync.dma_start(xtm[i][:], x_d[ts(i, P), :])
            qkv0 = make_qkv(0)

            def ln1_tm(i):
                scr = xlp.tile([P, C], F32, tag="scr", bufs=1, name=f"scr{i}")
                cols = {}
                for nm in ("s", "q", "mu", "var", "musq", "sd", "inv", "c0"):
                    cols[nm] = spool.tile([P, 1], F32, tag="lncol", bufs=24,
                                          name=f"{nm}{i}")
                nc.vector.tensor_reduce(cols["s"][:], xtm[i][:], AX.X, ALU.add)
                nc.scalar.activation(scr[:], xtm[i][:], AF.Square,
                                     accum_out=cols["q"][:])
                nc.scalar.mul(cols["mu"][:], cols["s"][:], 1.0 / C)
                nc.scalar.mul(cols["var"][:], cols["q"][:], 1.0 / C)
                nc.vector.tensor_mul(cols["musq"][:], cols["mu"][:],
                                     cols["mu"][:])
                nc.vector.tensor_sub(cols["var"][:], cols["var"][:],
                                     cols["musq"][:])
                nc.scalar.activation(cols["sd"][:], cols["var"][:], AF.Sqrt,
                                     bias=eps_col[:])
                nc.vector.reciprocal(cols["inv"][:], cols["sd"][:])
                nc.vector.tensor_mul(cols["c0"][:], cols["mu"][:],
                                     cols["inv"][:])
                nc.scalar.mul(cols["c0"][:], cols["c0"][:], -1.0)
                nc.scalar.activation(htm[i][:], xtm[i][:], AF.Identity,
                                     bias=cols["c0"][:],
                                     scale=cols["inv"][:])

            def xtr(i):
                for g in range(2):
                    pst = ps_lin.tile([P, 4, P], F32R, tag="lin",
                                      name=f"trx{i}_{g}")
                    for mi in range(4):
                        nc.tensor.transpose(pst[:, mi, :],
                                            xtm[i][:, ts(4 * g + mi, P)],
                                            ident_r[:])
                    nc.vector.tensor_copy(xf[:, 4 * g:4 * g + 4, ts(i, P)],
                                          pst[:])

            for i in range(NT):
                xtr(i)
                ln1_tm(i)
                if i > 0:
                    tr_tile(h18[:, :, ts(i - 1, P)], htm[i - 1][:],
                            f"h{i - 1}", on_dve=(i % 2 == 0), evict_scale=SA)
                if i == 4:
                    # h1 rows for t=0 complete: overlap QKV(0) t=0 only
                    # (t=1 units would head-of-line block the in-order PE
                    # queue behind the not-yet-emitted transposes of 4..7)
                    for _ in range(12):
                        next(qkv0[3], None)
            tr_tile(h18[:, :, ts(NT - 1, P)], htm[NT - 1][:], f"h{NT - 1}",
                    on_dve=False, evict_scale=SA)

        # ---------------- attention: pipelined per head-block ----------------
        with tc.tile_pool(name="ptp", bufs=1) as ptp, \
             tc.tile_pool(name="ypool", bufs=1) as ypool:

            yf = ypool.tile([P, NCH, T], BF16, tag="y", name="yf")

            def v_transpose(hb, v):
                va = qkvp.tile([P, NT, 130], BF16, tag="vaug", bufs=2,
                               name=f"va{hb}")
                pst = ps_lin.tile([P, NT, P], BF16, tag="lin", name=f"vtr{hb}")
                for ki in range(NT):
                    nc.tensor.transpose(pst[:, ki, :], v[:, ts(ki, P)],
                                        ident_b[:])
                dst = va[:, :, 0:130].rearrange(
                    "p k (h c) -> p k h c", h=2)[:, :, :, 0:64]
                src = pst[:].rearrange("p k (h c) -> p k h c", h=2)
                nc.vector.tensor_copy(dst, src)
                nc.vector.tensor_copy(
                    va[:, :, 64:65].rearrange("p k o -> p (k o)"), ones8[:])
                nc.vector.tensor_copy(
                    va[:, :, 129:130].rearrange("p k o -> p (k o)"), ones8[:])
                return va

            def emit_st(hb, qi, ki, q, k):
                """One wide ST tile + exp (+mask); returns the P tile."""
                st = ps_st.tile([P, T], F32, tag="st", name=f"st{hb}_{qi}_{ki}")
                for p_ in range(2):
                    nc.tensor.matmul(
                        st[:, ts(p_, 512)],
                        k[p_ * 64:(p_ + 1) * 64, ts(ki, P)],
                        q[p_ * 64:(p_ + 1) * 64, ts(qi, 512)],
                        start=True, stop=True)
                pt = ptp.tile([P, T], BF16, tag="pt", bufs=12,
                              name=f"pt{hb}_{qi}_{ki}")
                nc.scalar.activation(pt[:], st[:], AF.Exp,
                                     bias=zero_col[:], scale=SCALE)
                d = ki - 4 * qi
                if d >= 0:
                    # NOTE: keep these on DVE — gpsimd adds ~1.2us semaphore
                    # latency per cross-engine handoff and stalls PV
                    for p_ in range(2):
                        nc.vector.tensor_mul(pt[:, ts(p_, 512)],
                                             pt[:, ts(p_, 512)], masks[d][:])
                return pt

            def make_pv(hb, qi, va, pts, out):
                """Generator: PV accumulation in 2-mm units, then dn chain."""
                def gen():
                    pvs = []
                    kmax = 4 * qi + 3
                    for p_ in range(2):
                        pv = ps_pv.tile([P, 512], F32, tag="pv",
                                        name=f"pv{hb}_{qi}_{p_}")
                        for ki in range(kmax + 1):
                            nc.tensor.matmul(
                                pv[0:65, :],
                                va[:, ki, p_ * 65:(p_ + 1) * 65],
                                pts[ki][:, ts(p_, 512)],
                                start=(ki == 0), stop=(ki == kmax))
                            if ki % 2 == 1:
                                yield
                        pvs.append(pv)
                    # raw denominators in f32r; the reciprocal runs wide
                    # (approx_fast — the accurate wide reciprocal is a
                    # ~6.5us multi-pass Newton sequence, measured) on the
                    # [64,T] broadcast in dn_bcast_finish
                    dnr = qkvp.tile([1, T], F32R, tag="dnrow", bufs=3,
                                    name=f"dr{hb}_{qi}")
                    for p_ in range(2):
                        nc.vector.tensor_copy(dnr[0:1, ts(p_, 512)],
                                              pvs[p_][64:65, :])
                    out.extend([pvs, dnr])
                return gen()

            def dn_bcast_finish(hb, qi, pvs, dnr, use_lin=False):
                """Broadcast raw denominators over 64 partitions, take the
                reciprocal wide, write y (bf16)."""
                dnb = qkvp.tile([64, T], F32, tag="dnb", bufs=1,
                                name=f"dnbs{hb}_{qi}")
                if use_lin:
                    for p_ in range(2):
                        bps = ps_lin.tile([P, 512], F32, tag="lin",
                                          name=f"dnb{hb}_{qi}_{p_}")
                        nc.tensor.matmul(bps[0:64, :], ones_row[:, 0:64],
                                         dnr[0:1, ts(p_, 512)],
                                         start=True, stop=True)
                        nc.vector.reciprocal_approx_fast(dnb[:, ts(p_, 512)],
                                                         bps[0:64, :])
                else:
                    bps = ps_st.tile([P, T], F32, tag="st",
                                     name=f"dnb{hb}_{qi}")
                    for p_ in range(2):
                        nc.tensor.matmul(bps[0:64, ts(p_, 512)],
                                         ones_row[:, 0:64],
                                         dnr[0:1, ts(p_, 512)],
                                         start=True, stop=True)
                    nc.vector.reciprocal_approx_fast(dnb[:], bps[0:64, :])
                for p_ in range(2):
                    nc.vector.tensor_mul(
                        yf[p_ * 64:(p_ + 1) * 64, hb, ts(qi, 512)],
                        pvs[p_][0:64, :], dnb[:, ts(p_, 512)])

            def pump(g, n=None):
                if g is None:
                    return True
                try:
                    if n is None:
                        while True:
                            next(g)
                    else:
                        for _ in range(n):
                            next(g)
                except StopIteration:
                    return True
                return False

            # --- pipelined head-block loop ---
            q, k, v, gq = qkv0
            pump(gq)                       # finish QKV(0) (partly ran in load)
            gq = None
            prev1 = None                   # (hb, va, pts1) -> PV in next iter
            pend0 = None                   # (hb, pvs0, dnr0) -> bcast next iter
            for hb in range(NCH):
                va = v_transpose(hb, v)
                if hb < NCH - 1:
                    qn, kn, vn, gq = make_qkv(hb + 1)
                else:
                    qn = kn = vn = gq = None
                res1 = []
                gpv1 = None
                pv1_done = prev1 is None
                if prev1 is not None:
                    phb, pva, ppts1 = prev1
                    gpv1 = make_pv(phb, 1, pva, ppts1, res1)
                res0 = []
                gpv0 = None
                pts0, pts1 = [], []
                st_items = [(0, ki) for ki in range(4)] + \
                           [(1, ki) for ki in range(NT)]
                for idx, (qi, ki) in enumerate(st_items):
                    pt = emit_st(hb, qi, ki, q, k)
                    (pts0 if qi == 0 else pts1).append(pt)
                    if idx == 0 and pend0 is not None:
                        dn_bcast_finish(pend0[0], 0, pend0[1], pend0[2])
                        pend0 = None
                    if gpv1 is None and not pv1_done:
                        dn_bcast_finish(phb, 1, res1[0], res1[1])
                        pv1_done = True
                    if idx == 7:
                        if gpv1 is not None:
                            pump(gpv1)
                            gpv1 = None
                        if not pv1_done:
                            dn_bcast_finish(phb, 1, res1[0], res1[1])
                            pv1_done = True
                        gpv0 = make_pv(hb, 0, va, pts0, res0)
                    for _ in range(3):
                        if gpv1 is not None:
                            if pump(gpv1, 1):
                                gpv1 = None
                        elif idx >= 8 and gpv0 is not None:
                            if pump(gpv0, 1):
                                gpv0 = None
                        elif gq is not None:
                            if pump(gq, 1):
                                gq = None
                pump(gpv0)
                pump(gq)
                gq = None
                pend0 = (hb, res0[0], res0[1])
                prev1 = (hb, va, pts1)
                q, k, v = qn, kn, vn

            # prefetch proj + FFN1-qtr0 weights before the leftovers
            ws = {}
            for m in range(2):
                w = wpool.tile([P, NCH, P], BF16, tag="wqkv", bufs=3,
                               name=f"wproj{m}_pre")
                nc.sync.dma_start(w[:], Wproj_d[m])
                ws[m] = w
            w1_pre, a1_pre = [], []
            for mm_ in range(8):
                wt = ffnp.tile([P, NCH, P], BF16, tag="w1", bufs=8,
                               name=f"w1_{mm_}")
                nc.sync.dma_start(wt[:], W1_d[mm_])
                a = ffnp.tile([P, T], BF16, tag="a1", bufs=8, name=f"a1_{mm_}")
                w1_pre.append(wt)
                a1_pre.append(a)
            w2_pre = {}
            for m in range(2):
                w2t = ffnp.tile([P, 8, P], BF16, tag="w2", bufs=3,
                                name=f"w2_0_{m}")
                nc.sync.dma_start(w2t[:], W2_d[m])
                w2_pre[m] = w2t

            # --- leftovers + proj + LN2 ---
            if pend0 is not None:
                dn_bcast_finish(pend0[0], 0, pend0[1], pend0[2])
            phb, pva, ppts1 = prev1
            res1 = []
            pump(make_pv(phb, 1, pva, ppts1, res1))
            # bproj pre-add into the residual (ACT; r1 = x + bproj + y@Wp)
            for m in range(NCH):
                nc.scalar.activation(xf[:, m, :], xf[:, m, :], AF.Identity,
                                     bias=bproj_t[:, m:m + 1], scale=1.0)

            ln2_stats = [ps_st.tile([1, T], F32, tag="st", name=f"ln2_st{t}")
                         for t in range(NQ)]

            def ln2_sq(m, t):
                sq = spool.tile([P, 512], BF16, tag="sq", bufs=2,
                                name=f"ln2sq{m}_{t}")
                nc.gpsimd.tensor_mul(sq[:], xf[:, m, ts(t, 512)],
                                     xf[:, m, ts(t, 512)])
                return sq

            def ln2_stat_mms(m, t, sq):
                nc.tensor.matmul(ln2_stats[t][0:1, 0:512],
                                 ones_col_b[:], xf[:, m, ts(t, 512)],
                                 start=(m == 0), stop=(m == NCH - 1))
                nc.tensor.matmul(ln2_stats[t][0:1, 512:1024],
                                 ones_col_b[:], sq[:],
                                 start=(m == 0), stop=(m == NCH - 1))

            def ln2_finalize(t):
                # broadcast the RAW stats across partitions first (PE), then
                # do all the math as wide [128,512] ops at full engine rate —
                # single-partition [1,512] DVE ops run one lane (~2.4us each)
                srow = spool.tile([1, T], F32R, tag="lnrows", bufs=1,
                                  name=f"ln2srow{t}")
                nc.scalar.activation(srow[:], ln2_stats[t][0:1, :], AF.Copy)
                bps = ps_st.tile([P, T], F32, tag="st", name=f"ln2bps{t}")
                for half in range(2):
                    nc.tensor.matmul(bps[:, ts(half, 512)], ones_row[:],
                                     srow[0:1, ts(half, 512)],
                                     start=True, stop=True)
                wt = lambda nm: spool.tile([P, 512], F32, tag="lnw", bufs=4,
                                           name=f"ln2{nm}{t}")
                mu_t, msq_t, var_t, c0f = wt("mu"), wt("msq"), wt("var"), \
                    wt("c0f")
                nc.scalar.mul(mu_t[:], bps[:, 0:512], 1.0 / C)
                nc.scalar.activation(msq_t[:], mu_t[:], AF.Square)
                nc.scalar.mul(var_t[:], bps[:, 512:1024], 1.0 / C)
                nc.vector.tensor_sub(var_t[:], var_t[:], msq_t[:])
                nc.scalar.activation(var_t[:], var_t[:], AF.Sqrt,
                                     bias=eps_col[:])
                # bf16 broadcast rows: the apply's mul runs at 2x DVE rate
                bc = spool.tile([P, T], BF16, tag="lnbc", bufs=2,
                                name=f"ln2bc{t}")
                nc.vector.reciprocal(bc[:, 0:512], var_t[:])
                nc.vector.tensor_mul(c0f[:], mu_t[:], bc[:, 0:512])
                nc.scalar.mul(bc[:, 512:1024], c0f[:], -1.0)
                return bc

            h2f = hpool.tile([P, NCH, T], BF16, tag="h", name="h2f")

            def ln2_apply(c, t, bc):
                # DVE staged via PSUM (3-SBUF-operand DVE ops run at 1/3
                # rate; gpsimd is far too slow for bulk elementwise work)
                ps = ps_pv.tile([P, 512], F32, tag="pv", name=f"ap{c}_{t}")
                nc.vector.tensor_mul(ps[:], xf[:, c, ts(t, 512)],
                                     bc[:, 0:512])
                nc.vector.tensor_add(h2f[:, c, ts(t, 512)], ps[:],
                                     bc[:, 512:1024])

            def proj_pass(t, first, bc_prev=None):
                def load(m):
                    if m < NCH and m not in ws:
                        w = wpool.tile([P, NCH, P], BF16, tag="wqkv", bufs=3,
                                       name=f"wproj{m}_{t}")
                        nc.sync.dma_start(w[:], Wproj_d[m])
                        ws[m] = w

                load(0)
                load(1)
                sqs = {}
                for m in range(NCH):
                    load(m + 2)
                    ps = ps_lin.tile([P, 512], F32, tag="lin",
                                     name=f"proj_ps{m}_{t}")
                    for j in range(NCH):
                        nc.tensor.matmul(ps[:], ws[m][:, j, :],
                                         yf[:, j, ts(t, 512)],
                                         start=(j == 0), stop=(j == NCH - 1))
                    if first and m == 0:
                        # finish (7, q1) while proj keeps the PE busy
                        dn_bcast_finish(phb, 1, res1[0], res1[1], use_lin=True)
                    nc.vector.tensor_add(xf[:, m, ts(t, 512)],
                                         xf[:, m, ts(t, 512)], ps[:])
                    # squares on gpsimd; stat matmuls lag one group so the PE
                    # never waits on the gpsimd queue
                    sqs[m] = ln2_sq(m, t)
                    if m > 0:
                        ln2_stat_mms(m - 1, t, sqs[m - 1])
                    if bc_prev is not None:
                        # interleave prev-t LN2 applies between this pass's
                        # DVE adds so neither chain delays the other's
                        # downstream consumers (stats t / FFN1 prev-t)
                        ln2_apply(m, 1 - t, bc_prev)
                ln2_stat_mms(NCH - 1, t, sqs[NCH - 1])
                if t == 0:
                    ws.clear()   # re-DMA per t (tiles recycled, bufs=3)

            proj_pass(0, True)
            bc0 = ln2_finalize(0)
            proj_pass(1, False, bc_prev=bc0)
            # finalize(1) / apply(1) / b2 pre-add are emitted inside the
            # FFN section, after the first FFN1 t=0 groups, so their
            # latency hides under PE matmul work

        # ---------------- FFN (4 d_ff quarters) + residual + out -------------
        qkv_ctx.close()
        if True:
            def ffn1_group(mg, t, a, wt):
                ps = ps_lin.tile([P, 512], F32, tag="lin",
                                 name=f"f1ps{mg}_{t}")
                for j in range(NCH):
                    nc.tensor.matmul(ps[:], wt[:, j, :],
                                     h2f[:, j, ts(t, 512)],
                                     start=(j == 0), stop=(j == NCH - 1))
                nc.scalar.activation(a[:, ts(t, 512)], ps[:], AF.Relu,
                                     bias=b1_t[:, mg:mg + 1], scale=1.0)

            w1_tiles = {0: w1_pre}
            for qtr in range(4):
                wts = w1_tiles.pop(qtr)
                if qtr == 0:
                    a1 = a1_pre
                else:
                    a1 = [ffnp.tile([P, T], BF16, tag="a1", bufs=8,
                                    name=f"a1_{qtr * 8 + mm_}")
                          for mm_ in range(8)]
                for mm_ in range(8):
                    ffn1_group(qtr * 8 + mm_, 0, a1[mm_], wts[mm_])
                if qtr == 0:
                    bc1 = ln2_finalize(1)
                    for c in range(NCH):
                        ln2_apply(c, 1, bc1)   # DVE; overlaps FFN1 t=0
                    for m in range(NCH):       # pre-add b2 (stats read done)
                        nc.scalar.activation(xf[:, m, :], xf[:, m, :],
                                             AF.Identity,
                                             bias=b2_t[:, m:m + 1], scale=1.0)
                for mm_ in range(8):
                    ffn1_group(qtr * 8 + mm_, 1, a1[mm_], wts[mm_])
                if qtr + 1 < 4:
                    # issue next quarter's W1 DMAs now: the tile buffers'
                    # readers (this quarter's FFN1) are already emitted, so
                    # the DMAs stream in during FFN2
                    nxt = []
                    for mm_ in range(8):
                        mg = (qtr + 1) * 8 + mm_
                        wt = ffnp.tile([P, NCH, P], BF16, tag="w1", bufs=8,
                                       name=f"w1_{mg}")
                        nc.sync.dma_start(wt[:], W1_d[mg])
                        nxt.append(wt)
                    w1_tiles[qtr + 1] = nxt

                w2s = w2_pre if qtr == 0 else {}

                def load2(m, qtr=qtr, w2s=w2s):
                    if m < NCH and m not in w2s:
                        w2t = ffnp.tile([P, 8, P], BF16, tag="w2", bufs=3,
                                        name=f"w2_{qtr}_{m}")
                        nc.sync.dma_start(w2t[:], W2_d[qtr * 8 + m])
                        w2s[m] = w2t

                load2(0)
                load2(1)
                for m in range(NCH):
                    load2(m + 2)
                    for t in range(NQ):
                        ps = ps_lin.tile([P, 512], F32, tag="lin",
                                         name=f"f2ps{qtr}_{m}_{t}")
                        for j in range(8):
                            nc.tensor.matmul(ps[:], w2s[m][:, j, :],
                                             a1[j][:, ts(t, 512)],
                                             start=(j == 0), stop=(j == 7))
                        nc.vector.tensor_add(xf[:, m, ts(t, 512)],
                                             xf[:, m, ts(t, 512)], ps[:])
                    if qtr == 3:
                        # xf[:, m] final: per-half transpose + evict + DMA
                        # so the last output DMA starts as early as possible
                        om = ffnp.tile([P, T], F32, tag="om", bufs=2,
                                       name=f"om{m}")
                        omr = om[:].rearrange("p (g i f) -> p g i f",
                                              g=2, i=4)
                        dst = out_d[:, ts(m, P)].rearrange(
                            "(g i p) f -> g p i f", g=2, p=P)
                        pst = ps_lin.tile([P, NT, P], BF16, tag="lin",
                                          name=f"otr{m}")
                        for g in range(2):
                            for ii in range(4):
                                nc.tensor.transpose(
                                    pst[:, 4 * g + ii, :],
                                    xf[:, m, ts(4 * g + ii, P)], ident_b[:])
                            if m % 2 == 0:
                                nc.scalar.activation(
                                    omr[:, g], pst[:, 4 * g:4 * g + 4, :],
                                    AF.Copy)
                            else:
                                nc.vector.tensor_copy(
                                    omr[:, g], pst[:, 4 * g:4 * g + 4, :])
                            nc.sync.dma_start(dst[g], omr[:, g])

    nc.compile()
    return nc


_NC_CACHE = {}


def _get_nc():
    if "nc" not in _NC_CACHE:
        _NC_CACHE["nc"] = _build()
    return _NC_CACHE["nc"]


def _fold_inputs(inputs):
    """Fold LN gains/biases into downstream weights; cast bf16; repack
    tile-major so each [128, kchunks, 128] weight tile is one contiguous
    DMA."""
    import ml_dtypes

    f = lambda kk: np.asarray(inputs[kk], dtype=np.float32)
    Wqkv, bqkv = f("Wqkv"), f("bqkv")
    W1, b1 = f("W1"), f("b1")
    ln1_g, ln1_b = f("ln1_g"), f("ln1_b")
    ln2_g, ln2_b = f("ln2_g"), f("ln2_b")

    def pack(w):
        # [K, M] -> [M/128 tiles, 128 kpart, K/128 kchunk, 128 mcol]
        K, M = w.shape
        t = w.reshape(K // P, P, M // P, P).transpose(2, 1, 0, 3)
        return np.ascontiguousarray(t.astype(ml_dtypes.bfloat16))

    def pack_fp8_dr(w, sa):
        # per-output-chunk e4m3 quantization + DoubleRow k-pair layout:
        # [K, M] -> [M/128 tiles, 128 kpart, K/256 pairs, 2, 128 mcol];
        # returns (tiles, dequant vector [M] = 1/(sa*sw_chunk))
        K, M = w.shape
        nt = M // P
        sw = 240.0 / np.abs(w.reshape(K, nt, P)).max(axis=(0, 2))  # [nt]
        w8 = np.clip(w.reshape(K, nt, P) * sw[None, :, None],
                     -240, 240).astype(ml_dtypes.float8_e4m3fn)
        t = w8.reshape(K // 256, 2, P, nt, P).transpose(3, 2, 0, 1, 4)
        dq = np.repeat(1.0 / (sa * sw), P).astype(np.float32)
        return np.ascontiguousarray(t), np.ascontiguousarray(dq)

    def pack_w2(w):
        # [4096, 1024] -> [(qtr m) tiles, 128, 8 kchunk-in-qtr, 128]
        K, M = w.shape
        t = w.reshape(4, 8, P, M // P, P)          # qtr, j, p, m, c
        t = t.transpose(0, 3, 2, 1, 4).reshape(4 * (M // P), P, 8, P)
        return np.ascontiguousarray(t.astype(ml_dtypes.bfloat16))

    Wq8, qdq = pack_fp8_dr(ln1_g[:, None] * Wqkv, 16.0)
    return {
        "Wqkv": Wq8,
        "qdq": qdq,
        "bqkv": np.ascontiguousarray(bqkv + ln1_b @ Wqkv),
        "Wproj": pack(f("Wproj")),
        "bproj": np.ascontiguousarray(f("bproj")),
        "W1": pack(ln2_g[:, None] * W1),
        "b1": np.ascontiguousarray(b1 + ln2_b @ W1),
        "W2": pack_w2(f("W2")),
        "b2": np.ascontiguousarray(f("b2")),
    }


def kernel(**inputs):
    from concourse.bass_utils import run_bass_kernel_spmd

    nc = _get_nc()
    shared = _fold_inputs(inputs)
    x = np.asarray(inputs["x"], dtype=np.float32)
    in_maps = [dict(shared, x=np.ascontiguousarray(x[i])) for i in range(B)]
    res = run_bass_kernel_spmd(nc, in_maps, core_ids=list(range(B)))
    out = np.stack([res.results[i]["out"] for i in range(B)], axis=0)
    return out.astype(np.float32)
